# revision 8
# baseline (speedup 1.0000x reference)
"""Multi-head attention forward (b=8, n=2048, dim=512, heads=8, dh=64) on 8
Trainium2 NeuronCores.

Sharding: data-parallel over batch — core i computes the full attention layer
for batch element i (weights replicated, no collectives).

Per-core pipeline (everything "transposed" so softmax rowsums come out of the
same matmuls):
  1. x [2048,512] -> PE-transpose -> xT [512,2048]
  2. qkT = w_qk.T @ xT           [1024,2048]  (q/k features on partitions)
  3. v   = x @ w_v               [2048,512]   (tokens on partitions) + ones col
  4. per head h, per 512-wide query block i:
       simT[j,i]  = kT_h[:,j].T @ qT_h[:,i]      (K=64)
       expT       = exp(SCALE*simT)              (ACT)
       outT_aug   = sum_j v_aug[j].T @ expT      (K=128; row 64 = exp rowsum)
       rinv       = 1/rowsum; broadcast over 64 partitions via K=1 matmul
       attn_outT[hd, i] = outT_aug[0:64] * rinv_bcast
  5. out = attn_outT.T @ w_out   [2048,512]

Matmuls run as float32r (1 cycle/row vs 4 for float32). The hardware requires
f32r matmul operands to be *produced* rounded, so every SBUF tile feeding a
matmul is allocated as float32r and written by a rounding op (copy/activation).
"""

import os

import numpy as np

import concourse.bass as bass
import concourse.mybir as mybir
import concourse.tile as tile
from concourse import bacc
from concourse.masks import make_identity

FP32 = mybir.dt.float32
F32R = mybir.dt.float32r

B = 8
N = 2048
D = 512
H = 8
DH = 64
F3 = 3 * D
SCALE = DH**-0.5
P = 128
NT = N // P  # 16 token tiles
CT = D // P  # 4 contraction tiles over dim
NB = N // 512  # 4 query blocks of 512
JT = N // P  # 16 key tiles

# float32r streams at 1 cycle/row (vs 4 for float32) when the moving free dim
# is >=256; numerics are close to fp32. Set BASS_ATTN_FP32=1 to fall back.
_USE_F32R = os.environ.get("BASS_ATTN_FP32", "0") != "1"
MM_DT = F32R if _USE_F32R else FP32


def _attention_body(tc: "tile.TileContext"):
    nc = tc.nc
    x = nc.dram_tensor("x", [N, D], FP32, kind="ExternalInput").ap()
    w_qkv = nc.dram_tensor("w_qkv", [D, F3], FP32, kind="ExternalInput").ap()
    w_out = nc.dram_tensor("w_out", [D, D], FP32, kind="ExternalInput").ap()
    out = nc.dram_tensor("out", [N, D], FP32, kind="ExternalOutput").ap()

    exp_f = mybir.ActivationFunctionType.Exp

    with (
        tc.tile_pool(name="const", bufs=1) as const,
        tc.tile_pool(name="persist", bufs=1) as persist,
        tc.tile_pool(name="wstage", bufs=2) as wstage,
    ):
        identity = const.tile([P, P], FP32)
        make_identity(nc, identity)
        # memset can't write f32r; build ones in fp32 and round via copies
        ones32 = const.tile([P, 1], FP32)
        nc.vector.memset(ones32, 1.0)
        ones_1x64 = const.tile([1, 64], MM_DT)
        nc.vector.tensor_copy(out=ones_1x64, in_=ones32[0:1, :].to_broadcast([1, 64]))

        # weights: DMA fp32 -> rounding copy into MM_DT tiles
        wout_sb = persist.tile([P, CT, D], MM_DT)
        for t in range(CT):
            if _USE_F32R:
                ws = wstage.tile([P, F3], FP32, tag="ws")
                nc.sync.dma_start(out=ws[:, :D], in_=w_out[t * P : (t + 1) * P, :])
                nc.vector.tensor_copy(out=wout_sb[:, t, :], in_=ws[:, :D])
            else:
                nc.sync.dma_start(
                    out=wout_sb[:, t, :], in_=w_out[t * P : (t + 1) * P, :]
                )

        # q and k features transposed: rows = 1024 q/k features in 8 tiles
        qkT = persist.tile([P, 8, N], MM_DT)
        # v with tokens on partitions; per head 64 value cols + 1 ones col
        v_aug = persist.tile([P, JT, H * 65], MM_DT)
        nc.vector.tensor_copy(
            out=v_aug.rearrange("p j (h c) -> p j h c", c=65)[:, :, :, 64:65],
            in_=ones32.to_broadcast([P, JT, H, 1]),
        )

        with (
            tc.tile_pool(name="proj", bufs=1) as proj_pool,
            tc.tile_pool(name="xstage", bufs=3) as xstage,
            tc.tile_pool(name="pst", bufs=4, space="PSUM") as pst,
            tc.tile_pool(name="psmm", bufs=4, space="PSUM") as psmm,
        ):
            wqkv_sb = proj_pool.tile([P, CT, F3], MM_DT)
            for t in range(CT):
                if _USE_F32R:
                    ws = wstage.tile([P, F3], FP32, tag="ws")
                    nc.sync.dma_start(out=ws, in_=w_qkv[t * P : (t + 1) * P, :])
                    nc.vector.tensor_copy(out=wqkv_sb[:, t, :], in_=ws)
                else:
                    nc.sync.dma_start(
                        out=wqkv_sb[:, t, :], in_=w_qkv[t * P : (t + 1) * P, :]
                    )

            # ---- load x and transpose to xT [512, 2048] ----
            xT = proj_pool.tile([P, CT, N], MM_DT)
            for j in range(NT):
                xs = xstage.tile([P, D], FP32)
                nc.sync.dma_start(out=xs, in_=x[j * P : (j + 1) * P, :])
                for t in range(CT):
                    ps = pst.tile([P, P], FP32)
                    nc.tensor.transpose(ps, xs[:, t * P : (t + 1) * P], identity)
                    nc.vector.tensor_copy(out=xT[:, t, j * P : (j + 1) * P], in_=ps)

            # ---- qkT = w_qk.T @ xT ----
            for m in range(8):
                for nb in range(NB):
                    ps = psmm.tile([P, 512], FP32, tag="mm")
                    for c in range(CT):
                        nc.tensor.matmul(
                            ps,
                            wqkv_sb[:, c, m * P : (m + 1) * P],
                            xT[:, c, nb * 512 : (nb + 1) * 512],
                            start=(c == 0),
                            stop=(c == CT - 1),
                        )
                    nc.vector.tensor_copy(
                        out=qkT[:, m, nb * 512 : (nb + 1) * 512], in_=ps
                    )

            # ---- v = x @ w_v (tokens on partitions) ----
            for j in range(NT):
                ps = psmm.tile([P, 512], FP32, tag="mm")
                for c in range(CT):
                    nc.tensor.matmul(
                        ps,
                        xT[:, c, j * P : (j + 1) * P],
                        wqkv_sb[:, c, 2 * D : 3 * D],
                        start=(c == 0),
                        stop=(c == CT - 1),
                    )
                nc.vector.tensor_copy(
                    out=v_aug[:, j, :].rearrange("p (h c) -> p h c", c=65)[:, :, 0:64],
                    in_=ps.rearrange("p (h c) -> p h c", c=64),
                )

        # ---- attention per head ----
        with (
            tc.tile_pool(name="attno", bufs=1) as attno_pool,
            tc.tile_pool(name="expp", bufs=4) as expp,
            tc.tile_pool(name="rinvp", bufs=4) as rinvp,
            tc.tile_pool(name="outstage", bufs=3) as outstage,
            tc.tile_pool(name="pss", bufs=4, space="PSUM") as pssp,
            tc.tile_pool(name="pso", bufs=2, space="PSUM") as psop,
            tc.tile_pool(name="psb", bufs=2, space="PSUM") as psbp,
        ):
            attn_outT = attno_pool.tile([P, CT, N], MM_DT)
            for h in range(H):
                qt, qo = h // 2, (h % 2) * 64
                kt = 4 + h // 2
                for ib in range(NB):
                    isl = slice(ib * 512, (ib + 1) * 512)
                    pso = psop.tile([P, 512], FP32)
                    for j in range(JT):
                        pss = pssp.tile([P, 512], FP32, tag="mm")
                        nc.tensor.matmul(
                            pss,
                            qkT[qo : qo + 64, kt, j * P : (j + 1) * P],
                            qkT[qo : qo + 64, qt, isl],
                            start=True,
                            stop=True,
                        )
                        ex = expp.tile([P, 512], MM_DT)
                        nc.scalar.activation(out=ex, in_=pss, func=exp_f, scale=SCALE)
                        nc.tensor.matmul(
                            pso[0:65, :],
                            v_aug[:, j, h * 65 : (h + 1) * 65],
                            ex,
                            start=(j == 0),
                            stop=(j == JT - 1),
                        )
                    rinv = rinvp.tile([1, 512], MM_DT, tag="rinv")
                    with nc.allow_low_precision("f32r rounding of softmax recip"):
                        nc.vector.reciprocal(out=rinv, in_=pso[64:65, :])
                    psb = psbp.tile([64, 512], FP32)
                    nc.tensor.matmul(psb, ones_1x64, rinv, start=True, stop=True)
                    rb = rinvp.tile([64, 512], FP32, tag="rb")
                    nc.vector.tensor_copy(out=rb, in_=psb)
                    with nc.allow_low_precision("f32r rounding of attn out"):
                        nc.vector.tensor_mul(
                            out=attn_outT[qo : qo + 64, qt, isl],
                            in0=pso[0:64, :],
                            in1=rb,
                        )

            # ---- out = attn_outT.T @ w_out ----
            for j in range(NT):
                ps = pssp.tile([P, 512], FP32, tag="mm")
                for t in range(CT):
                    nc.tensor.matmul(
                        ps,
                        attn_outT[:, t, j * P : (j + 1) * P],
                        wout_sb[:, t, :],
                        start=(t == 0),
                        stop=(t == CT - 1),
                    )
                os_ = outstage.tile([P, D], FP32)
                nc.vector.tensor_copy(out=os_, in_=ps)
                nc.sync.dma_start(out=out[j * P : (j + 1) * P, :], in_=os_)


_CACHE: dict = {}


def build_nc() -> "bass.Bass":
    if "nc" not in _CACHE:
        nc = bacc.Bacc("TRN2", target_bir_lowering=False, debug=False)
        with tile.TileContext(nc) as tc:
            _attention_body(tc)
        nc.compile()
        _CACHE["nc"] = nc
    return _CACHE["nc"]


def kernel(x: np.ndarray, w_qkv: np.ndarray, w_out: np.ndarray) -> np.ndarray:
    from concourse.bass_utils import run_bass_kernel_spmd

    nc = build_nc()
    x = np.ascontiguousarray(np.asarray(x, dtype=np.float32))
    w_qkv = np.ascontiguousarray(np.asarray(w_qkv, dtype=np.float32))
    w_out = np.ascontiguousarray(np.asarray(w_out, dtype=np.float32))
    in_maps = [
        {"x": x[i], "w_qkv": w_qkv, "w_out": w_out} for i in range(B)
    ]
    res = run_bass_kernel_spmd(nc, in_maps, core_ids=list(range(B)))
    return np.stack([r["out"] for r in res.results], axis=0)
